# revision 1
# baseline (speedup 1.0000x reference)
"""DenseMRConv (gnn message passing) on 8 TRN2 NeuronCores via Bass/Tile.

Math (reference):
    x_j  = x[edge_index]                      # [N, K, d] gather
    diff = max_k(x_j - x_i) = max_k(x_j) - x  # max distributes over const
    out  = concat([x, diff]) @ W + b
         = x @ (W_top - W_bot) + max_k(x_j) @ W_bot + b

Sharding: nodes (rows of x / edge_index) are split across the 8 cores;
x is replicated on every core as the gather table (edge_index addresses
global node ids); the small MLP weights are replicated.

Per 128-node tile on each core:
  1. one indirect DMA gathers all 128*32 neighbor rows into [128, 32*64]
  2. DVE in-place halving tree -> M = max_k x_j  [128, 64]
  3. PE transposes x_tile and M (feat-major needed for matmul contraction)
  4. PE: psum = xT.T @ (W_top-W_bot) + MT.T @ W_bot + ones.T @ b
  5. result tile DMAs back to DRAM
"""

import numpy as np

N, K, D, DOUT = 100000, 32, 64, 64
N_CORES = 8
P = 128
SHARD = N // N_CORES            # 12500 nodes per core
TILES = (SHARD + P - 1) // P    # 98
SHARD_PAD = TILES * P           # 12544 (remainder tile padded w/ index 0)

TRACE = False                   # test.py sets True to collect HW exec time
LAST_EXEC_TIME_NS = None

_CACHE = {}


def _build():
    import concourse.bacc as bacc
    import concourse.bass as bass
    import concourse.mybir as mybir
    import concourse.tile as tile
    from concourse.masks import make_identity

    f32 = mybir.dt.float32
    i32 = mybir.dt.int32

    nc = bacc.Bacc("TRN2", target_bir_lowering=False, debug=False,
                   num_devices=N_CORES)

    xg_d = nc.dram_tensor("xg", [N, D], f32, kind="ExternalInput")
    xs_d = nc.dram_tensor("xs", [SHARD_PAD, D], f32, kind="ExternalInput")
    ei_d = nc.dram_tensor("ei", [SHARD_PAD, K], i32, kind="ExternalInput")
    a_d = nc.dram_tensor("a", [D, DOUT], f32, kind="ExternalInput")   # W_top - W_bot
    wb_d = nc.dram_tensor("wb", [D, DOUT], f32, kind="ExternalInput")  # W_bot
    b_d = nc.dram_tensor("b", [1, DOUT], f32, kind="ExternalInput")
    out_d = nc.dram_tensor("out", [SHARD_PAD, DOUT], f32, kind="ExternalOutput")

    # tiled DRAM views: [tile, partition, ...]
    xs_t = xs_d.ap().rearrange("(t p) d -> t p d", p=P)
    ei_t = ei_d.ap().rearrange("(t p) k -> t p k", p=P)
    out_t = out_d.ap().rearrange("(t p) d -> t p d", p=P)

    with tile.TileContext(nc) as tc:
        with (
            tc.tile_pool(name="const", bufs=1) as cpool,
            tc.tile_pool(name="gather", bufs=10) as gpool,
            tc.tile_pool(name="small", bufs=8) as spool,
            tc.tile_pool(name="psum", bufs=2, space="PSUM") as ppool,
        ):
            ident = cpool.tile([P, P], f32)
            make_identity(nc, ident[:])
            ones1 = cpool.tile([1, P], f32)
            nc.gpsimd.memset(ones1[:], 1.0)
            a_t = cpool.tile([D, DOUT], f32)
            nc.sync.dma_start(a_t[:], a_d.ap())
            wb_t = cpool.tile([D, DOUT], f32)
            nc.sync.dma_start(wb_t[:], wb_d.ap())
            b_t = cpool.tile([1, DOUT], f32)
            nc.sync.dma_start(b_t[:], b_d.ap())

            for t in range(TILES):
                idx = spool.tile([P, K], i32, tag="idx")
                nc.sync.dma_start(idx[:], ei_t[t])

                g = gpool.tile([P, K * D], f32, tag="g")
                # HW indirect-DMA semantics: ONE index per partition; each
                # partition receives `dest free run` consecutive elements
                # starting at idx*D.  So issue one gather per neighbor slot.
                for k in range(K):
                    nc.gpsimd.indirect_dma_start(
                        out=g[:, k * D:(k + 1) * D], out_offset=None,
                        in_=xg_d.ap(),
                        in_offset=bass.IndirectOffsetOnAxis(ap=idx[:, k:k + 1], axis=0),
                    )

                # in-place halving max tree: K*D -> D
                w = K * D
                while w > D:
                    w //= 2
                    nc.vector.tensor_tensor(
                        out=g[:, :w], in0=g[:, :w], in1=g[:, w:2 * w],
                        op=mybir.AluOpType.max,
                    )
                # M = g[:, :D]

                xt = spool.tile([P, D], f32, tag="xt")
                nc.sync.dma_start(xt[:], xs_t[t])

                xT_p = ppool.tile([D, P], f32, tag="xT")
                nc.tensor.transpose(out=xT_p[:], in_=xt[:], identity=ident[:])
                mT_p = ppool.tile([D, P], f32, tag="mT")
                nc.tensor.transpose(out=mT_p[:], in_=g[:, :D], identity=ident[:])

                xT = spool.tile([D, P], f32, tag="xTs")
                nc.vector.tensor_copy(out=xT[:], in_=xT_p[:])
                mT = spool.tile([D, P], f32, tag="mTs")
                nc.vector.tensor_copy(out=mT[:], in_=mT_p[:])

                o_p = ppool.tile([P, DOUT], f32, tag="o")
                nc.tensor.matmul(o_p[:], lhsT=xT[:], rhs=a_t[:],
                                 start=True, stop=False)
                nc.tensor.matmul(o_p[:], lhsT=mT[:], rhs=wb_t[:],
                                 start=False, stop=False)
                nc.tensor.matmul(o_p[:], lhsT=ones1[:], rhs=b_t[:],
                                 start=False, stop=True)

                o_s = spool.tile([P, DOUT], f32, tag="os")
                nc.vector.tensor_copy(out=o_s[:], in_=o_p[:])
                nc.sync.dma_start(out_t[t], o_s[:])

    nc.compile()
    return nc


def _prep_inputs(x, edge_index, W, b):
    x = np.ascontiguousarray(np.asarray(x, dtype=np.float32))
    ei = np.asarray(edge_index).astype(np.int32)
    W = np.asarray(W, dtype=np.float32)
    b = np.asarray(b, dtype=np.float32).reshape(1, DOUT)
    A = np.ascontiguousarray(W[:D] - W[D:])
    Wb = np.ascontiguousarray(W[D:])

    in_maps = []
    for c in range(N_CORES):
        lo = c * SHARD
        xs = np.zeros((SHARD_PAD, D), np.float32)
        xs[:SHARD] = x[lo:lo + SHARD]
        eis = np.zeros((SHARD_PAD, K), np.int32)
        eis[:SHARD] = ei[lo:lo + SHARD]
        in_maps.append({
            "xg": x, "xs": xs, "ei": eis, "a": A, "wb": Wb, "b": b,
        })
    return in_maps


def _install_trace_shim():
    """Provide antenv.axon_hooks (missing in this image) so
    run_bass_kernel_spmd(trace=True) can collect an NTFF profile."""
    import sys
    import types
    try:
        from antenv import axon_hooks  # noqa: F401
        return
    except ImportError:
        pass
    import antenv
    from concourse import bass_utils
    mod = types.ModuleType("antenv.axon_hooks")
    _hook = [None]
    mod.set_axon_ntff_profile_hook = lambda h: _hook.__setitem__(0, h)
    mod.get_axon_ntff_profile_hook = lambda: _hook[0]
    sys.modules["antenv.axon_hooks"] = mod
    antenv.axon_hooks = mod
    from trn_agent_boot.trn_boot import _ntff_profile_via_ctypes
    mod.set_axon_ntff_profile_hook(
        _ntff_profile_via_ctypes("/opt/axon/libaxon_pjrt.so"))
    bass_utils.upload_artifacts = lambda d: d


def kernel(x, edge_index, W, b):
    global LAST_EXEC_TIME_NS
    from concourse import bass_utils

    if TRACE:
        _install_trace_shim()

    if "nc" not in _CACHE:
        _CACHE["nc"] = _build()
    nc = _CACHE["nc"]

    in_maps = _prep_inputs(x, edge_index, W, b)
    res = bass_utils.run_bass_kernel_spmd(
        nc, in_maps, core_ids=list(range(N_CORES)), trace=TRACE,
    )
    LAST_EXEC_TIME_NS = res.exec_time_ns
    out = np.concatenate(
        [res.results[c]["out"][:SHARD] for c in range(N_CORES)], axis=0
    )
    return out.astype(np.float32)


if __name__ == "__main__":
    rng = np.random.default_rng(0)
    x = rng.standard_normal((N, D), dtype=np.float32)
    ei = rng.integers(0, N, (N, K)).astype(np.int64)
    W = (rng.standard_normal((2 * D, DOUT)) / np.sqrt(2 * D)).astype(np.float32)
    b = np.zeros(DOUT, np.float32)
    out = kernel(x, ei, W, b)
    M = np.max(x[ei], axis=1)
    exp = x @ (W[:D] - W[D:]) + M @ W[D:] + b
    err = np.abs(out - exp).max() / np.abs(exp).max()
    print("rel err:", err)

